# revision 10
# baseline (speedup 1.0000x reference)
"""GCN encoder (2x GCNConv + linear projection, relu) on 8 Trainium2 cores.

v2 design (vs v1 baseline):
  - Layer 1 does NO table build and NO dma_gather: the gathered operand is
    raw x rows (known at staging time), so the host pre-expands the per-edge
    message stream into a dense [nchunk1*128, C] bf16 input per core
    (pure indexing/duplication/cast — no host FP math).  The device streams
    it sequentially over HWDGE at full HBM bandwidth, zero descriptors of
    Q7 work, starting at t=0.
  - Aggregate-then-transform for L1: psT = sum_e x[src]*dinv_src (+ diag
    self term), then dinv_d, then @W1.T — valid by linearity.
  - Transposed (feature-major) window outputs [C, 128]: no TensorE
    transposes; per-partition biases ride on ScalarE activation; the G2
    table matmul consumes out1T directly as lhsT, producing node-major
    table rows for the AllGather.
  - Layer 2 gathers from the AllGathered table g2d with dma_gather
    (per-desc packets, 4 SWDGE queues); descriptor generation for the
    first PREP_N pieces happens during L1 (prepare_only), triggered right
    after the AllGather.
  - Fused sel build: one DVE tensor_scalar (iota is_equal rel)*dinvsrc per
    L1 chunk; plain is_equal for L2 (table rows pre-scaled by dinv).

Math identity per GCNConv layer (deg = indeg(dst)+1, dinv = rsqrt(deg)):
    layer1: out1 = relu(W1 @ (dinv_d * (sum_{e->d} x[s]*dinv_s
                                        + x[d]*dinv_d)) + b1)
    layer2: table g2 = dinv * (out1 @ W2.T) (allgathered), then analogous.
"""

import sys
import numpy as np

for _p in ("/opt/trn_rl_repo",):
    if _p not in sys.path:
        sys.path.append(_p)

import concourse.bacc as bacc
import concourse.tile as tile
from concourse import bass, mybir, bass_utils

F32 = mybir.dt.float32
BF16 = mybir.dt.bfloat16
I16 = mybir.dt.int16
AF = mybir.ActivationFunctionType
ALU = mybir.AluOpType
NP_BF16 = mybir.dt.np(BF16)
FP8 = mybir.dt.float8e4
NP_FP8 = mybir.dt.np(FP8)


class Cfg:
    def __init__(self, n_nodes, n_edges, cores=8, in_c=128, hid_c=128, out_c=64):
        assert in_c == 128 and hid_c == 128
        self.N, self.E, self.CORES = n_nodes, n_edges, cores
        self.C, self.OUT_C = in_c, out_c
        assert n_nodes % cores == 0
        self.S = n_nodes // cores                       # real nodes per shard
        self.SP = -(-self.S // 128) * 128               # padded shard rows
        assert self.SP > self.S, "need pad rows in each shard for zero rows"
        self.NPAD = self.SP * cores                     # padded table rows
        assert self.NPAD % 256 == 0
        self.HALF = self.NPAD // 2                      # int16 table split
        assert self.HALF % self.SP == 0
        assert self.HALF < 32768
        self.NW = self.SP // 128                        # windows per core
        self.NW1 = (self.NW + 1) // 2                   # windows in shard-half A
        self.RH1 = self.NW1 * 128                       # rows in shard-half A
        self.RH2 = self.SP - self.RH1
        assert cores * self.RH1 < 32768 and cores * self.RH2 < 32768
        self.GBLK = 16                                  # L2 gather blocks/piece
        self.PREP_N = 0                                # L2 pieces prepped early


CFG = Cfg(50000, 800000)


def _wrap16(a):
    """[L] -> [128, L/16] int16 idx layout for dma_gather (16-wrap, 8x repl)."""
    assert a.size % 16 == 0
    w = a.reshape(-1, 16).T.astype(np.int16)
    return np.ascontiguousarray(np.tile(w, (8, 1)))


def _host_prep(cfg, x, edge_index):
    """Build per-core device inputs + the compile-time chunk schedule."""
    N, C = cfg.N, cfg.C
    S, SP, NPAD, HALF, NW, CORES = cfg.S, cfg.SP, cfg.NPAD, cfg.HALF, cfg.NW, cfg.CORES

    src = np.asarray(edge_index[0]).astype(np.int64)
    dst = np.asarray(edge_index[1]).astype(np.int64)
    deg = np.bincount(dst, minlength=N).astype(np.float32) + 1.0

    RH1, RH2 = cfg.RH1, cfg.RH2
    owner = dst // S
    loc = dst - owner * S
    srcp = (src // S) * SP + (src % S)          # padded global src id
    win = loc // 128
    rel = (loc % 128).astype(np.float32)
    srcc = srcp // SP
    lp = srcp - srcc * SP                       # local row within src shard
    hB = lp >= RH1                              # shard-half of the src row
    gidx = np.where(hB, srcc * RH2 + (lp - RH1), srcc * RH1 + lp)

    # L1 (dense stream): chunks per (window) only
    cnt1 = np.bincount(owner * NW + win, minlength=CORES * NW).reshape(CORES, NW)
    caps1 = -(-cnt1.max(axis=0) // 128)                 # [NW]
    # L2 (gather): chunks per (window, half)
    key = (owner * NW + win) * 2 + hB
    counts = np.bincount(key, minlength=CORES * NW * 2).reshape(CORES, NW, 2)
    maxc = counts.max(axis=0)                           # [NW, 2]
    capA = -(-maxc[:, 0] // 128)
    capB = -(-maxc[:, 1] // 128)
    glenA = -(-maxc[:, 0] // 16) * 16
    glenB = -(-maxc[:, 1] // 16) * 16

    nodes = np.arange(N, dtype=np.int64)
    realpos = (nodes // S) * SP + (nodes % S)
    degp = np.ones(NPAD, np.float32)
    degp[realpos] = deg

    xpad = np.zeros((NPAD, C), np.float32)
    xpad[realpos] = np.asarray(x, np.float32)
    xrb = xpad.astype(NP_BF16)                                    # [NPAD, C]

    # pad entries: contributions are killed by rel=-1 selection; half-B can
    # point at a guaranteed-zero pad row, half-A at row 0 (finite garbage).
    ZROWA, ZROWB = 0, S - RH1
    nchunk1 = int(caps1.sum())

    per_core = []
    for c in range(CORES):
        m = owner == c
        cw, cr, cs, ch, cg = win[m], rel[m], srcp[m], hB[m], gidx[m]

        # ---- L1 dense message stream (window-major, no half split) ----
        o1 = np.argsort(cw, kind="stable")
        w1_, r1_, s1_ = cw[o1], cr[o1], cs[o1]
        msgstream = np.zeros((nchunk1 * 128, C), NP_BF16)
        rel1 = np.full(nchunk1 * 128, -1.0, np.float32)
        dsrc1 = np.ones(nchunk1 * 128, np.float32)
        cb = 0
        for wi in range(NW):
            lo = np.searchsorted(w1_, wi, "left")
            hi = np.searchsorted(w1_, wi, "right")
            n = hi - lo
            assert n <= caps1[wi] * 128
            msgstream[cb * 128:cb * 128 + n] = xrb[s1_[lo:hi]]
            rel1[cb * 128:cb * 128 + n] = r1_[lo:hi]
            dsrc1[cb * 128:cb * 128 + n] = degp[s1_[lo:hi]]
            cb += caps1[wi]
        msgstream = np.ascontiguousarray(
            msgstream.reshape(nchunk1, 128, C).transpose(1, 0, 2)
            .reshape(128, nchunk1 * C))
        rr = rel1.astype(np.int64)
        mk = rr >= 0
        sel1s = np.zeros((nchunk1 * 128, 128), NP_FP8)
        sel1s[np.nonzero(mk)[0], rr[mk]] = 1.0
        sel1s = np.ascontiguousarray(
            sel1s.reshape(nchunk1, 128, 128).transpose(1, 0, 2)
            .reshape(128, nchunk1 * 128))
        dsrc1T = np.ascontiguousarray(dsrc1.reshape(-1, 128).T)

        # ---- L2 gather lists ((window, half)-major) ----
        o2 = np.lexsort((ch, cw))
        cw2, cr2, cg2, ch2 = cw[o2], cr[o2], cg[o2], ch[o2]
        k2 = cw2 * 2 + ch2
        ia_parts, ib_parts, rel_parts = [], [], []
        for wi in range(NW):
            for half, cap, glen in ((0, capA[wi], glenA[wi]),
                                    (1, capB[wi], glenB[wi])):
                lo = np.searchsorted(k2, wi * 2 + half, "left")
                hi = np.searchsorted(k2, wi * 2 + half, "right")
                n = hi - lo
                assert n <= glen <= cap * 128
                zp = ZROWB if half else ZROWA
                iv = np.concatenate(
                    [cg2[lo:hi], np.full(glen - n, zp, np.int64)])
                rv = np.concatenate(
                    [cr2[lo:hi], np.full(cap * 128 - n, -1.0, np.float32)])
                (ib_parts if half else ia_parts).append(iv)
                rel_parts.append(rv)
        idxa = np.concatenate(ia_parts) if ia_parts else np.zeros(0, np.int64)
        idxb = np.concatenate(ib_parts) if ib_parts else np.zeros(0, np.int64)
        rel2_all = np.concatenate(rel_parts).astype(np.float32)
        rel2T = np.ascontiguousarray(
            rel2_all.reshape(-1, 128).T).astype(NP_BF16)

        degl = np.ones((128, 128), np.float32)
        degl[:, :NW] = degp[c * SP:(c + 1) * SP].reshape(NW, 128).T
        degdt = np.ascontiguousarray(np.tile(
            degp[c * SP:(c + 1) * SP][None, :], (128, 1)))         # [128, SP]
        xloc = np.ascontiguousarray(xrb[c * SP:(c + 1) * SP, :])   # [SP, C]
        per_core.append(dict(
            idxa=_wrap16(idxa), idxb=_wrap16(idxb), rel2=rel2T,
            msgstream=msgstream, sel1s=sel1s, degsrc1=dsrc1T,
            degl=np.ascontiguousarray(degl), degdt=degdt, xloc=xloc))

    sched = dict(caps1=[int(v) for v in caps1],
                 capA=[int(v) for v in capA], capB=[int(v) for v in capB],
                 glenA=[int(v) for v in glenA], glenB=[int(v) for v in glenB])
    return sched, per_core


def _build_nc(cfg, sched):
    C, OUT_C = cfg.C, cfg.OUT_C
    SP, NPAD, NW, CORES = cfg.SP, cfg.NPAD, cfg.NW, cfg.CORES
    NW1, RH1, RH2 = cfg.NW1, cfg.RH1, cfg.RH2
    caps1 = sched["caps1"]
    capA, capB = sched["capA"], sched["capB"]
    glenA, glenB = sched["glenA"], sched["glenB"]
    nchunk1 = sum(caps1)
    nchunk2 = sum(capA) + sum(capB)
    la16 = sum(glenA) // 16
    lb16 = sum(glenB) // 16
    GBLK = cfg.GBLK
    gmaxblk = max(
        [min(GBLK, -(-g // 128)) for g in glenA + glenB if g] or [1])
    maxcap1 = max(caps1)

    nc = bacc.Bacc("TRN2", target_bir_lowering=False, debug=False,
                   enable_asserts=False, num_devices=CORES,
                   num_swdge_queues=4)

    def inp(name, shape, dt=F32):
        return nc.dram_tensor(name, shape, dt, kind="ExternalInput").ap()

    ms_d = inp("msgstream", [128, nchunk1 * C], BF16)
    xloc_d = inp("xloc", [SP, C], BF16)
    w1t_d = inp("w1t", [C, C], BF16)
    w2t_d = inp("w2t", [C, C], BF16)
    wpt_d = inp("wpt", [C, OUT_C], BF16)
    b1c_d = inp("b1c", [128, 1])
    b2c_d = inp("b2c", [128, 1])
    bpc_d = inp("bpc", [OUT_C, 1])
    degl_d = inp("degl", [128, 128])
    degsrc1_d = inp("degsrc1", [128, nchunk1])
    degdt_d = inp("degdt", [128, SP])
    iotar_d = inp("iotar", [128, 32 * 128], BF16)
    identb_d = inp("identb", [128, 128], BF16)
    idxa_d = inp("idxa", [128, max(la16, 16)], I16)
    idxb_d = inp("idxb", [128, max(lb16, 16)], I16)
    sel1s_d = inp("sel1s", [128, nchunk1 * 128], FP8)
    rel2_d = inp("rel2", [128, nchunk2], BF16)
    out_d = nc.dram_tensor("out", [OUT_C, SP], F32, kind="ExternalOutput").ap()

    g2locA = nc.dram_tensor("g2locA", [RH1, C], BF16, kind="Internal").ap()
    g2locB = nc.dram_tensor("g2locB", [RH2, C], BF16, kind="Internal").ap()
    g2dA = nc.dram_tensor("g2dA", [CORES * RH1, C], BF16, kind="Internal",
                          addr_space="Shared").ap()
    g2dB = nc.dram_tensor("g2dB", [CORES * RH2, C], BF16, kind="Internal",
                          addr_space="Shared").ap()

    from contextlib import ExitStack
    with tile.TileContext(nc) as tc, ExitStack() as ctx:
        cp = ctx.enter_context(tc.tile_pool(name="consts", bufs=1))
        locp = ctx.enter_context(tc.tile_pool(name="xlocs", bufs=1))
        msp = ctx.enter_context(tc.tile_pool(name="ms", bufs=3))
        msgp = ctx.enter_context(tc.tile_pool(name="msg", bufs=12))
        prepp = ctx.enter_context(tc.tile_pool(name="prep", bufs=1))
        spool = ctx.enter_context(tc.tile_pool(name="sel", bufs=3))
        epool = ctx.enter_context(tc.tile_pool(name="epi", bufs=4))
        gpool = ctx.enter_context(tc.tile_pool(name="g2b", bufs=1))
        apool = ctx.enter_context(tc.tile_pool(name="aggA", bufs=1))
        ppool_w = ctx.enter_context(tc.tile_pool(name="psw", bufs=3, space="PSUM"))
        ppool_wb = ctx.enter_context(tc.tile_pool(name="pswb", bufs=2, space="PSUM"))
        ppool_h = ctx.enter_context(tc.tile_pool(name="psh", bufs=1, space="PSUM"))
        ppool_g = ctx.enter_context(tc.tile_pool(name="psg", bufs=1, space="PSUM"))
        ppool_p = ctx.enter_context(tc.tile_pool(name="psp", bufs=1, space="PSUM"))

        def cload(name, ap, shape, dt=F32):
            t = cp.tile(shape, dt, tag=name)
            nc.sync.dma_start(t[:], ap[:])
            return t

        w1t = cload("w1t", w1t_d, [C, C], BF16)
        w2t = cload("w2t", w2t_d, [C, C], BF16)
        wpt = cload("wpt", wpt_d, [C, OUT_C], BF16)
        b1c = cload("b1c", b1c_d, [128, 1])
        b2c = cload("b2c", b2c_d, [128, 1])
        bpc = cload("bpc", bpc_d, [OUT_C, 1])
        degl = cload("degl", degl_d, [128, 128])
        degsrc1 = cload("degsrc1", degsrc1_d, [128, nchunk1])
        degdt = cload("degdt", degdt_d, [128, SP])
        iotar = cload("iotar", iotar_d, [128, 32 * 128], BF16)
        identb = cload("identb", identb_d, [128, 128], BF16)
        idxa = cload("idxa", idxa_d, [128, max(la16, 16)], I16)
        idxb = cload("idxb", idxb_d, [128, max(lb16, 16)], I16)
        rel2 = cload("rel2", rel2_d, [128, nchunk2], BF16)

        # ---- dinv computations (rsqrt activation banned for accuracy) ----
        sql = cp.tile([128, 128], F32, tag="sql")
        nc.scalar.activation(sql[:], degl[:], AF.Sqrt)
        dinvl = cp.tile([128, 128], F32, tag="dinvl")
        nc.vector.reciprocal_approx_fast(dinvl[:], sql[:])

        sqs = cp.tile([128, nchunk1], F32, tag="sqs")
        nc.scalar.activation(sqs[:], degsrc1[:], AF.Sqrt)
        dinvsrc1f = cp.tile([128, nchunk1], F32, tag="dinvsrc1f")
        nc.vector.reciprocal_approx_fast(dinvsrc1f[:], sqs[:])
        dinvsrc1 = cp.tile([128, nchunk1], BF16, tag="dinvsrc1")
        nc.vector.tensor_copy(dinvsrc1[:], dinvsrc1f[:])

        # dinvdtile[p, d] = dinv_dst[d] (degdt input is already partition-
        # broadcast on host; rsqrt it here: Sqrt into a scratch, reciprocal
        # back into the degdt tile which becomes dinvdtile)
        sqd = cp.tile([128, SP], F32, tag="sqd")
        nc.scalar.activation(sqd[:], degdt[:], AF.Sqrt)
        dinvdtile = degdt
        nc.vector.reciprocal_approx_fast(dinvdtile[:], sqd[:])

        # ---- local x tiles (node-major) for the L1 self term ----
        xloc_tiles = []
        for w in range(NW):
            xt_ = locp.tile([128, C], BF16, tag=f"xloc_{w}")
            nc.sync.dma_start(xt_[:], xloc_d[w * 128:(w + 1) * 128, :])
            xloc_tiles.append(xt_)

        # ---- L2 gather piece enumeration ----
        pieces = []
        offa = offb = 0   # in idx columns (16 idx each)
        cbase = 0
        for w in range(NW):
            for half, cap, glen in ((0, capA[w], glenA[w]),
                                    (1, capB[w], glenB[w])):
                if cap == 0:
                    continue
                hbase = cbase + (capA[w] if half else 0)
                gleft = glen
                for g0 in range(0, cap, GBLK):
                    gb = min(GBLK, cap - g0)
                    nidx = min(gleft, gb * 128)
                    gleft -= nidx
                    assert nidx > 0
                    off = offa if half == 0 else offb
                    pieces.append((w, half, nidx, off, hbase + g0))
                    if half == 0:
                        offa += -(-nidx // 16)
                    else:
                        offb += -(-nidx // 16)
            cbase += capA[w] + capB[w]

        def emit_gather(piece, qi, pool, tag):
            w, half, nidx, off, _ci = piece
            nblk = -(-nidx // 128)
            msg = pool.tile([128, gmaxblk, C], BF16, tag=tag)
            isl = (idxa if half == 0 else idxb)[:, off:off + -(-nidx // 16)]
            tab = g2dA[:] if half == 0 else g2dB[:]
            nc.gpsimd.dma_gather(msg[:, :nblk, :], tab, isl, nidx, nidx,
                                 elem_size=C, single_packet=False,
                                 queue_num=qi % 4)
            return msg

        selw = max(maxcap1, gmaxblk)
        assert selw <= 32

        def iotav(k):
            return iotar[:, :k * 128].rearrange("p (c d) -> p c d", d=128)

        # ================= layer 1 (dense stream) =================
        g2b_tiles = []
        cb = 0
        for w in range(NW):
            cap = caps1[w]
            ps = ppool_w.tile([128, 128], F32, tag="pswA")
            diag = epool.tile([128, 128], BF16, tag="diag")
            nc.scalar.activation(diag[:], identb[:], AF.Identity,
                                 scale=dinvl[:, w:w + 1])
            nc.tensor.matmul(ps[:], lhsT=xloc_tiles[w][:], rhs=diag[:],
                             start=True, stop=False)
            mst = msp.tile([128, maxcap1, C], BF16, tag="mst")
            nc.sync.dma_start(
                mst[:, :cap, :],
                ms_d[:, cb * C:(cb + cap) * C].rearrange(
                    "p (j f) -> p j f", f=C))
            # fp8-staged one-hot sel stream (0/1 exact in fp8);
            # the dinv_src multiply upcasts to bf16 for the matmul.
            s1t = spool.tile([128, selw, 128], FP8, tag="s1t")
            nc.sync.dma_start(
                s1t[:, :cap, :],
                sel1s_d[:, cb * 128:(cb + cap) * 128].rearrange(
                    "p (j d) -> p j d", d=128))
            selc = spool.tile([128, selw, 128], BF16, tag="selc")
            nc.vector.tensor_tensor(
                out=selc[:, :cap, :], in0=s1t[:, :cap, :],
                in1=dinvsrc1[:, cb:cb + cap].to_broadcast([128, cap, 128]),
                op=ALU.mult)
            for k in range(cap):
                nc.tensor.matmul(ps[:], lhsT=mst[:, k, :], rhs=selc[:, k, :],
                                 start=False, stop=(k == cap - 1))
            cb += cap
            # epilogue: out1T = relu(W1 @ (dinv_d * psT) + b1)
            s1 = epool.tile([128, 128], BF16, tag="s1")
            nc.vector.tensor_tensor(
                out=s1[:], in0=ps[:],
                in1=dinvdtile[:, w * 128:(w + 1) * 128], op=ALU.mult)
            psh = ppool_h.tile([128, 128], F32, tag="psh")
            nc.tensor.matmul(psh[:], lhsT=w1t[:], rhs=s1[:],
                             start=True, stop=True)
            o1 = epool.tile([128, 128], BF16, tag="o1")
            nc.scalar.activation(o1[:], psh[:], AF.Relu, bias=b1c[:, 0:1])
            # G2 row block: g2 = dinv_d * (out1 @ W2.T), node-major
            psg = ppool_g.tile([128, C], F32, tag="psg")
            nc.tensor.matmul(psg[:], lhsT=o1[:], rhs=w2t[:],
                             start=True, stop=True)
            g2b = gpool.tile([128, C], BF16, tag=f"g2b_{w}")
            nc.scalar.activation(g2b[:], psg[:], AF.Identity,
                                 scale=dinvl[:, w:w + 1])
            if w < NW1:
                nc.sync.dma_start(g2locA[w * 128:(w + 1) * 128, :], g2b[:])
            else:
                nc.sync.dma_start(
                    g2locB[(w - NW1) * 128:(w - NW1 + 1) * 128, :], g2b[:])
            g2b_tiles.append(g2b)
            if w == NW1 - 1:
                # shard-half A is complete on every core at roughly the same
                # time; gather it while the second half is still computing.
                nc.gpsimd.collective_compute(
                    "AllGather", ALU.bypass,
                    replica_groups=[list(range(CORES))],
                    ins=[g2locA[:]], outs=[g2dA[:]])

        # ================= layer 2 (gather, two passes) =================
        # pass A: shard-half-A contributions only (g2dA is available while
        # the second half of L1 still runs); spill partials to SBUF.
        piecesbyw = [([], []) for _ in range(NW)]
        for piece in pieces:
            piecesbyw[piece[0]][piece[1]].append(piece)
        qi = 0
        aggA_tiles = []
        for w in range(NW):
            apieces = piecesbyw[w][0]
            assert apieces
            ps = ppool_w.tile([128, 128], F32, tag="pswA")
            total_k = sum(-(-p[2] // 128) for p in apieces)
            done = 0
            for piece in apieces:
                _, half, nidx, off, ci0 = piece
                msg = emit_gather(piece, qi, msgp, "msg2")
                qi += 1
                nblk = -(-nidx // 128)
                selb = spool.tile([128, selw, 128], BF16, tag="selb")
                nc.vector.tensor_tensor(
                    out=selb[:, :nblk, :], in0=iotav(nblk),
                    in1=rel2[:, ci0:ci0 + nblk].to_broadcast(
                        [128, nblk, 128]),
                    op=ALU.is_equal)
                for k in range(nblk):
                    kk = min(128, nidx - k * 128)
                    done += 1
                    nc.tensor.matmul(ps[:], lhsT=msg[:kk, k, :],
                                     rhs=selb[:kk, k, :],
                                     start=(done == 1),
                                     stop=(done == total_k))
            sA = apool.tile([128, 128], BF16, tag=f"aggA_{w}")
            nc.vector.tensor_copy(sA[:], ps[:])
            aggA_tiles.append(sA)

        nc.gpsimd.collective_compute(
            "AllGather", ALU.bypass,
            replica_groups=[list(range(CORES))],
            ins=[g2locB[:]], outs=[g2dB[:]])

        # pass B: restore partials, add self term + shard-half-B chunks,
        # then epilogue + projection.
        for w in range(NW):
            bpieces = piecesbyw[w][1]
            assert bpieces
            ps = ppool_wb.tile([128, 128], F32, tag="pswB")
            nc.tensor.matmul(ps[:], lhsT=g2b_tiles[w][:], rhs=identb[:],
                             start=True, stop=False)
            nc.tensor.matmul(ps[:], lhsT=identb[:], rhs=aggA_tiles[w][:],
                             start=False, stop=False)
            total_k = sum(-(-p[2] // 128) for p in bpieces)
            done = 0
            for piece in bpieces:
                _, half, nidx, off, ci0 = piece
                msg = emit_gather(piece, qi, msgp, "msg2")
                qi += 1
                nblk = -(-nidx // 128)
                selb = spool.tile([128, selw, 128], BF16, tag="selb")
                nc.vector.tensor_tensor(
                    out=selb[:, :nblk, :], in0=iotav(nblk),
                    in1=rel2[:, ci0:ci0 + nblk].to_broadcast(
                        [128, nblk, 128]),
                    op=ALU.is_equal)
                for k in range(nblk):
                    kk = min(128, nidx - k * 128)
                    done += 1
                    nc.tensor.matmul(ps[:], lhsT=msg[:kk, k, :],
                                     rhs=selb[:kk, k, :],
                                     start=False, stop=(done == total_k))
            # epilogue: out2T = relu(dinv_d * psT + b2), then projection
            s2 = epool.tile([128, 128], BF16, tag="s2")
            nc.vector.tensor_tensor(
                out=s2[:], in0=ps[:],
                in1=dinvdtile[:, w * 128:(w + 1) * 128], op=ALU.mult)
            o2 = epool.tile([128, 128], BF16, tag="o2")
            nc.scalar.activation(o2[:], s2[:], AF.Relu, bias=b2c[:, 0:1])
            psp = ppool_p.tile([OUT_C, 128], F32, tag="psp")
            nc.tensor.matmul(psp[:], lhsT=wpt[:], rhs=o2[:],
                             start=True, stop=True)
            ofr = epool.tile([OUT_C, 128], F32, tag="ofr")
            nc.scalar.activation(ofr[:], psp[:], AF.Relu, bias=bpc[:, 0:1])
            nc.sync.dma_start(out_d[:, w * 128:(w + 1) * 128], ofr[:])

    nc.compile()
    return nc


def _make_in_maps(cfg, sched, per_core, W1, b1, W2, b2, Wp, bp):
    w1t = np.ascontiguousarray(np.asarray(W1, np.float32).T).astype(NP_BF16)
    w2t = np.ascontiguousarray(np.asarray(W2, np.float32).T).astype(NP_BF16)
    wpt = np.ascontiguousarray(np.asarray(Wp, np.float32).T).astype(NP_BF16)
    b1col = np.ascontiguousarray(np.asarray(b1, np.float32)[:, None])
    b2col = np.ascontiguousarray(np.asarray(b2, np.float32)[:, None])
    bpcol = np.ascontiguousarray(np.asarray(bp, np.float32)[:, None])
    iotar = np.tile(np.arange(128, dtype=np.float32)[None],
                    (128, 32)).astype(NP_BF16)
    identb = np.eye(128, dtype=np.float32).astype(NP_BF16)
    base = dict(w1t=w1t, w2t=w2t, wpt=wpt, b1c=b1col, b2c=b2col, bpc=bpcol,
                iotar=iotar, identb=identb)
    in_maps = []
    for c in range(cfg.CORES):
        pc = per_core[c]
        m = dict(base)
        m["msgstream"] = pc["msgstream"]
        m["sel1s"] = pc["sel1s"]
        m["degsrc1"] = pc["degsrc1"]
        m["idxa"] = pc["idxa"] if pc["idxa"].size else np.zeros((128, 16), np.int16)
        m["idxb"] = pc["idxb"] if pc["idxb"].size else np.zeros((128, 16), np.int16)
        m["rel2"] = pc["rel2"]
        m["degl"] = pc["degl"]
        m["degdt"] = pc["degdt"]
        m["xloc"] = pc["xloc"]
        in_maps.append(m)
    return in_maps


def _run(inputs, cfg=None, trace=False, tmpdir=None, verbose=True):
    import time
    t0 = time.time()
    def _log(msg):
        if verbose:
            print(f"[kernel {time.time()-t0:7.1f}s] {msg}", flush=True)
    cfg = cfg or CFG
    sched, per_core = _host_prep(cfg, inputs["x"], inputs["edge_index"])
    _log("host prep done")
    nc = _build_nc(cfg, sched)
    _log("build+compile done")
    in_maps = _make_in_maps(cfg, sched, per_core,
                            inputs["W1"], inputs["b1"], inputs["W2"],
                            inputs["b2"], inputs["Wp"], inputs["bp"])
    _log("in_maps done")
    core_ids = list(range(cfg.CORES))
    if trace:
        bass_utils.run_bass_kernel_spmd(nc, in_maps, core_ids=core_ids,
                                        trace=False)
        _log("warmup run done")
    res = bass_utils.run_bass_kernel_spmd(nc, in_maps, core_ids=core_ids,
                                          trace=trace, tmpdir=tmpdir)
    _log("run done")
    out = np.empty((cfg.N, cfg.OUT_C), np.float32)
    for c in range(cfg.CORES):
        out[c * cfg.S:(c + 1) * cfg.S] = res.results[c]["out"][:, :cfg.S].T
    return out, res


def kernel(**inputs):
    out, _ = _run(inputs)
    return out


# revision 12
# speedup vs baseline: 1.0151x; 1.0151x over previous
"""GCN encoder (2x GCNConv + linear projection, relu) on 8 Trainium2 cores.

v2 design (vs v1 baseline):
  - Layer 1 does NO table build and NO dma_gather: the gathered operand is
    raw x rows (known at staging time), so the host pre-expands the per-edge
    message stream into a dense [nchunk1*128, C] bf16 input per core
    (pure indexing/duplication/cast — no host FP math).  The device streams
    it sequentially over HWDGE at full HBM bandwidth, zero descriptors of
    Q7 work, starting at t=0.
  - Aggregate-then-transform for L1: psT = sum_e x[src]*dinv_src (+ diag
    self term), then dinv_d, then @W1.T — valid by linearity.
  - Transposed (feature-major) window outputs [C, 128]: no TensorE
    transposes; per-partition biases ride on ScalarE activation; the G2
    table matmul consumes out1T directly as lhsT, producing node-major
    table rows for the AllGather.
  - Layer 2 gathers from the AllGathered table g2d with dma_gather
    (per-desc packets, 4 SWDGE queues); descriptor generation for the
    first PREP_N pieces happens during L1 (prepare_only), triggered right
    after the AllGather.
  - Fused sel build: one DVE tensor_scalar (iota is_equal rel)*dinvsrc per
    L1 chunk; plain is_equal for L2 (table rows pre-scaled by dinv).

Math identity per GCNConv layer (deg = indeg(dst)+1, dinv = rsqrt(deg)):
    layer1: out1 = relu(W1 @ (dinv_d * (sum_{e->d} x[s]*dinv_s
                                        + x[d]*dinv_d)) + b1)
    layer2: table g2 = dinv * (out1 @ W2.T) (allgathered), then analogous.
"""

import sys
import numpy as np

for _p in ("/opt/trn_rl_repo",):
    if _p not in sys.path:
        sys.path.append(_p)

import concourse.bacc as bacc
import concourse.tile as tile
from concourse import bass, mybir, bass_utils

F32 = mybir.dt.float32
BF16 = mybir.dt.bfloat16
I16 = mybir.dt.int16
AF = mybir.ActivationFunctionType
ALU = mybir.AluOpType
NP_BF16 = mybir.dt.np(BF16)
FP8 = mybir.dt.float8e4
NP_FP8 = mybir.dt.np(FP8)


class Cfg:
    def __init__(self, n_nodes, n_edges, cores=8, in_c=128, hid_c=128, out_c=64):
        assert in_c == 128 and hid_c == 128
        self.N, self.E, self.CORES = n_nodes, n_edges, cores
        self.C, self.OUT_C = in_c, out_c
        assert n_nodes % cores == 0
        self.S = n_nodes // cores                       # real nodes per shard
        self.SP = -(-self.S // 128) * 128               # padded shard rows
        assert self.SP > self.S, "need pad rows in each shard for zero rows"
        self.NPAD = self.SP * cores                     # padded table rows
        assert self.NPAD % 256 == 0
        self.HALF = self.NPAD // 2                      # int16 table split
        assert self.HALF % self.SP == 0
        assert self.HALF < 32768
        self.NW = self.SP // 128                        # windows per core
        self.NW1 = (self.NW + 1) // 2                   # windows in shard-half A
        self.RH1 = self.NW1 * 128                       # rows in shard-half A
        self.RH2 = self.SP - self.RH1
        assert cores * self.RH1 < 32768 and cores * self.RH2 < 32768
        self.GBLK = 16                                  # L2 gather blocks/piece
        self.PREP_N = 0                                # L2 pieces prepped early


CFG = Cfg(50000, 800000)


def _wrap16(a):
    """[L] -> [128, L/16] int16 idx layout for dma_gather (16-wrap, 8x repl)."""
    assert a.size % 16 == 0
    w = a.reshape(-1, 16).T.astype(np.int16)
    return np.ascontiguousarray(np.tile(w, (8, 1)))


def _host_prep(cfg, x, edge_index):
    """Build per-core device inputs + the compile-time chunk schedule."""
    N, C = cfg.N, cfg.C
    S, SP, NPAD, HALF, NW, CORES = cfg.S, cfg.SP, cfg.NPAD, cfg.HALF, cfg.NW, cfg.CORES

    src = np.asarray(edge_index[0]).astype(np.int64)
    dst = np.asarray(edge_index[1]).astype(np.int64)
    deg = np.bincount(dst, minlength=N).astype(np.float32) + 1.0

    RH1, RH2 = cfg.RH1, cfg.RH2
    owner = dst // S
    loc = dst - owner * S
    srcp = (src // S) * SP + (src % S)          # padded global src id
    win = loc // 128
    rel = (loc % 128).astype(np.float32)
    srcc = srcp // SP
    lp = srcp - srcc * SP                       # local row within src shard
    hB = lp >= RH1                              # shard-half of the src row
    gidx = np.where(hB, srcc * RH2 + (lp - RH1), srcc * RH1 + lp)

    # L1 (dense stream): chunks per (window) only
    cnt1 = np.bincount(owner * NW + win, minlength=CORES * NW).reshape(CORES, NW)
    caps1 = -(-cnt1.max(axis=0) // 128)                 # [NW]
    # L2 (gather): chunks per (window, half)
    key = (owner * NW + win) * 2 + hB
    counts = np.bincount(key, minlength=CORES * NW * 2).reshape(CORES, NW, 2)
    maxc = counts.max(axis=0)                           # [NW, 2]
    capA = -(-maxc[:, 0] // 128)
    capB = -(-maxc[:, 1] // 128)
    glenA = -(-maxc[:, 0] // 16) * 16
    glenB = -(-maxc[:, 1] // 16) * 16

    nodes = np.arange(N, dtype=np.int64)
    realpos = (nodes // S) * SP + (nodes % S)
    degp = np.ones(NPAD, np.float32)
    degp[realpos] = deg

    xpad = np.zeros((NPAD, C), np.float32)
    xpad[realpos] = np.asarray(x, np.float32)
    xrb = xpad.astype(NP_BF16)                                    # [NPAD, C]

    # pad entries: contributions are killed by rel=-1 selection; half-B can
    # point at a guaranteed-zero pad row, half-A at row 0 (finite garbage).
    ZROWA, ZROWB = 0, S - RH1
    nchunk1 = int(caps1.sum())

    per_core = []
    for c in range(CORES):
        m = owner == c
        cw, cr, cs, ch, cg = win[m], rel[m], srcp[m], hB[m], gidx[m]

        # ---- L1 dense message stream (window-major, no half split) ----
        o1 = np.argsort(cw, kind="stable")
        w1_, r1_, s1_ = cw[o1], cr[o1], cs[o1]
        msgstream = np.zeros((nchunk1 * 128, C), NP_BF16)
        rel1 = np.full(nchunk1 * 128, -1.0, np.float32)
        dsrc1 = np.ones(nchunk1 * 128, np.float32)
        cb = 0
        for wi in range(NW):
            lo = np.searchsorted(w1_, wi, "left")
            hi = np.searchsorted(w1_, wi, "right")
            n = hi - lo
            assert n <= caps1[wi] * 128
            msgstream[cb * 128:cb * 128 + n] = xrb[s1_[lo:hi]]
            rel1[cb * 128:cb * 128 + n] = r1_[lo:hi]
            dsrc1[cb * 128:cb * 128 + n] = degp[s1_[lo:hi]]
            cb += caps1[wi]
        msgstream = np.ascontiguousarray(
            msgstream.reshape(nchunk1, 128, C).transpose(1, 0, 2)
            .reshape(128, nchunk1 * C))
        rr = rel1.astype(np.int64)
        mk = rr >= 0
        sel1s = np.zeros((nchunk1 * 128, 128), NP_FP8)
        sel1s[np.nonzero(mk)[0], rr[mk]] = 1.0
        sel1s = np.ascontiguousarray(
            sel1s.reshape(nchunk1, 128, 128).transpose(1, 0, 2)
            .reshape(128, nchunk1 * 128))
        dsrc1T = np.ascontiguousarray(dsrc1.reshape(-1, 128).T)

        # ---- L2 gather lists ((window, half)-major) ----
        o2 = np.lexsort((ch, cw))
        cw2, cr2, cg2, ch2 = cw[o2], cr[o2], cg[o2], ch[o2]
        k2 = cw2 * 2 + ch2
        ia_parts, ib_parts, rel_parts = [], [], []
        for wi in range(NW):
            for half, cap, glen in ((0, capA[wi], glenA[wi]),
                                    (1, capB[wi], glenB[wi])):
                lo = np.searchsorted(k2, wi * 2 + half, "left")
                hi = np.searchsorted(k2, wi * 2 + half, "right")
                n = hi - lo
                assert n <= glen <= cap * 128
                zp = ZROWB if half else ZROWA
                # ascending gather addresses within the segment help the
                # DRAM controller pipeline the latency-bound random reads
                so = np.argsort(cg2[lo:hi], kind="stable")
                iv = np.concatenate(
                    [cg2[lo:hi][so], np.full(glen - n, zp, np.int64)])
                rv = np.concatenate(
                    [cr2[lo:hi][so], np.full(cap * 128 - n, -1.0, np.float32)])
                (ib_parts if half else ia_parts).append(iv)
                rel_parts.append(rv)
        idxa = np.concatenate(ia_parts) if ia_parts else np.zeros(0, np.int64)
        idxb = np.concatenate(ib_parts) if ib_parts else np.zeros(0, np.int64)
        rel2_all = np.concatenate(rel_parts).astype(np.float32)
        rel2T = np.ascontiguousarray(
            rel2_all.reshape(-1, 128).T).astype(NP_BF16)

        degl = np.ones((128, 128), np.float32)
        degl[:, :NW] = degp[c * SP:(c + 1) * SP].reshape(NW, 128).T
        degdt = np.ascontiguousarray(np.tile(
            degp[c * SP:(c + 1) * SP][None, :], (128, 1)))         # [128, SP]
        xloc = np.ascontiguousarray(xrb[c * SP:(c + 1) * SP, :])   # [SP, C]
        per_core.append(dict(
            idxa=_wrap16(idxa), idxb=_wrap16(idxb), rel2=rel2T,
            msgstream=msgstream, sel1s=sel1s, degsrc1=dsrc1T,
            degl=np.ascontiguousarray(degl), degdt=degdt, xloc=xloc))

    sched = dict(caps1=[int(v) for v in caps1],
                 capA=[int(v) for v in capA], capB=[int(v) for v in capB],
                 glenA=[int(v) for v in glenA], glenB=[int(v) for v in glenB])
    return sched, per_core


def _build_nc(cfg, sched):
    C, OUT_C = cfg.C, cfg.OUT_C
    SP, NPAD, NW, CORES = cfg.SP, cfg.NPAD, cfg.NW, cfg.CORES
    NW1, RH1, RH2 = cfg.NW1, cfg.RH1, cfg.RH2
    caps1 = sched["caps1"]
    capA, capB = sched["capA"], sched["capB"]
    glenA, glenB = sched["glenA"], sched["glenB"]
    nchunk1 = sum(caps1)
    nchunk2 = sum(capA) + sum(capB)
    la16 = sum(glenA) // 16
    lb16 = sum(glenB) // 16
    GBLK = cfg.GBLK
    gmaxblk = max(
        [min(GBLK, -(-g // 128)) for g in glenA + glenB if g] or [1])
    maxcap1 = max(caps1)

    nc = bacc.Bacc("TRN2", target_bir_lowering=False, debug=False,
                   enable_asserts=False, num_devices=CORES,
                   num_swdge_queues=4)

    def inp(name, shape, dt=F32):
        return nc.dram_tensor(name, shape, dt, kind="ExternalInput").ap()

    ms_d = inp("msgstream", [128, nchunk1 * C], BF16)
    xloc_d = inp("xloc", [SP, C], BF16)
    w1t_d = inp("w1t", [C, C], BF16)
    w2t_d = inp("w2t", [C, C], BF16)
    wpt_d = inp("wpt", [C, OUT_C], BF16)
    b1c_d = inp("b1c", [128, 1])
    b2c_d = inp("b2c", [128, 1])
    bpc_d = inp("bpc", [OUT_C, 1])
    degl_d = inp("degl", [128, 128])
    degsrc1_d = inp("degsrc1", [128, nchunk1])
    degdt_d = inp("degdt", [128, SP])
    iotar_d = inp("iotar", [128, 32 * 128], BF16)
    identb_d = inp("identb", [128, 128], BF16)
    idxa_d = inp("idxa", [128, max(la16, 16)], I16)
    idxb_d = inp("idxb", [128, max(lb16, 16)], I16)
    sel1s_d = inp("sel1s", [128, nchunk1 * 128], FP8)
    rel2_d = inp("rel2", [128, nchunk2], BF16)
    out_d = nc.dram_tensor("out", [OUT_C, SP], F32, kind="ExternalOutput").ap()

    g2locA = nc.dram_tensor("g2locA", [RH1, C], BF16, kind="Internal").ap()
    g2locB = nc.dram_tensor("g2locB", [RH2, C], BF16, kind="Internal").ap()
    g2dA = nc.dram_tensor("g2dA", [CORES * RH1, C], BF16, kind="Internal",
                          addr_space="Shared").ap()
    g2dB = nc.dram_tensor("g2dB", [CORES * RH2, C], BF16, kind="Internal",
                          addr_space="Shared").ap()

    from contextlib import ExitStack
    with tile.TileContext(nc) as tc, ExitStack() as ctx:
        cp = ctx.enter_context(tc.tile_pool(name="consts", bufs=1))
        locp = ctx.enter_context(tc.tile_pool(name="xlocs", bufs=1))
        msp = ctx.enter_context(tc.tile_pool(name="ms", bufs=3))
        msgp = ctx.enter_context(tc.tile_pool(name="msg", bufs=12))
        prepp = ctx.enter_context(tc.tile_pool(name="prep", bufs=1))
        spool = ctx.enter_context(tc.tile_pool(name="sel", bufs=3))
        epool = ctx.enter_context(tc.tile_pool(name="epi", bufs=4))
        gpool = ctx.enter_context(tc.tile_pool(name="g2b", bufs=1))
        apool = ctx.enter_context(tc.tile_pool(name="aggA", bufs=1))
        ppool_w = ctx.enter_context(tc.tile_pool(name="psw", bufs=2, space="PSUM"))
        ppool_h = ctx.enter_context(tc.tile_pool(name="psh", bufs=1, space="PSUM"))
        ppool_g = ctx.enter_context(tc.tile_pool(name="psg", bufs=1, space="PSUM"))
        ppool_p = ctx.enter_context(tc.tile_pool(name="psp", bufs=1, space="PSUM"))

        def cload(name, ap, shape, dt=F32):
            t = cp.tile(shape, dt, tag=name)
            nc.sync.dma_start(t[:], ap[:])
            return t

        w1t = cload("w1t", w1t_d, [C, C], BF16)
        w2t = cload("w2t", w2t_d, [C, C], BF16)
        wpt = cload("wpt", wpt_d, [C, OUT_C], BF16)
        b1c = cload("b1c", b1c_d, [128, 1])
        b2c = cload("b2c", b2c_d, [128, 1])
        bpc = cload("bpc", bpc_d, [OUT_C, 1])
        degl = cload("degl", degl_d, [128, 128])
        degsrc1 = cload("degsrc1", degsrc1_d, [128, nchunk1])
        degdt = cload("degdt", degdt_d, [128, SP])
        iotar = cload("iotar", iotar_d, [128, 32 * 128], BF16)
        identb = cload("identb", identb_d, [128, 128], BF16)
        idxa = cload("idxa", idxa_d, [128, max(la16, 16)], I16)
        idxb = cload("idxb", idxb_d, [128, max(lb16, 16)], I16)
        rel2 = cload("rel2", rel2_d, [128, nchunk2], BF16)

        # ---- dinv computations (rsqrt activation banned for accuracy) ----
        sql = cp.tile([128, 128], F32, tag="sql")
        nc.scalar.activation(sql[:], degl[:], AF.Sqrt)
        dinvl = cp.tile([128, 128], F32, tag="dinvl")
        nc.vector.reciprocal_approx_fast(dinvl[:], sql[:])

        sqs = cp.tile([128, nchunk1], F32, tag="sqs")
        nc.scalar.activation(sqs[:], degsrc1[:], AF.Sqrt)
        dinvsrc1f = cp.tile([128, nchunk1], F32, tag="dinvsrc1f")
        nc.vector.reciprocal_approx_fast(dinvsrc1f[:], sqs[:])
        dinvsrc1 = cp.tile([128, nchunk1], BF16, tag="dinvsrc1")
        nc.vector.tensor_copy(dinvsrc1[:], dinvsrc1f[:])

        # dinvdtile[p, d] = dinv_dst[d] (degdt input is already partition-
        # broadcast on host; rsqrt it here: Sqrt into a scratch, reciprocal
        # back into the degdt tile which becomes dinvdtile)
        sqd = cp.tile([128, SP], F32, tag="sqd")
        nc.scalar.activation(sqd[:], degdt[:], AF.Sqrt)
        dinvdtile = degdt
        nc.vector.reciprocal_approx_fast(dinvdtile[:], sqd[:])

        # ---- local x tiles (node-major) for the L1 self term ----
        xloc_tiles = []
        for w in range(NW):
            xt_ = locp.tile([128, C], BF16, tag=f"xloc_{w}")
            nc.sync.dma_start(xt_[:], xloc_d[w * 128:(w + 1) * 128, :])
            xloc_tiles.append(xt_)

        # ---- L2 gather piece enumeration ----
        pieces = []
        offa = offb = 0   # in idx columns (16 idx each)
        cbase = 0
        for w in range(NW):
            for half, cap, glen in ((0, capA[w], glenA[w]),
                                    (1, capB[w], glenB[w])):
                if cap == 0:
                    continue
                hbase = cbase + (capA[w] if half else 0)
                gleft = glen
                for g0 in range(0, cap, GBLK):
                    gb = min(GBLK, cap - g0)
                    nidx = min(gleft, gb * 128)
                    gleft -= nidx
                    assert nidx > 0
                    off = offa if half == 0 else offb
                    pieces.append((w, half, nidx, off, hbase + g0))
                    if half == 0:
                        offa += -(-nidx // 16)
                    else:
                        offb += -(-nidx // 16)
            cbase += capA[w] + capB[w]

        def emit_gather(piece, qi, pool, tag):
            w, half, nidx, off, _ci = piece
            nblk = -(-nidx // 128)
            msg = pool.tile([128, gmaxblk, C], BF16, tag=tag)
            isl = (idxa if half == 0 else idxb)[:, off:off + -(-nidx // 16)]
            tab = g2dA[:] if half == 0 else g2dB[:]
            nc.gpsimd.dma_gather(msg[:, :nblk, :], tab, isl, nidx, nidx,
                                 elem_size=C, single_packet=False,
                                 queue_num=qi % 4)
            return msg

        selw = max(maxcap1, gmaxblk)
        assert selw <= 32

        def iotav(k):
            return iotar[:, :k * 128].rearrange("p (c d) -> p c d", d=128)

        # ================= layer 1 (dense stream) =================
        g2b_tiles = []
        cb = 0
        for w in range(NW):
            cap = caps1[w]
            ps = ppool_w.tile([128, 128], F32, tag="pswA")
            diag = epool.tile([128, 128], BF16, tag="diag")
            nc.scalar.activation(diag[:], identb[:], AF.Identity,
                                 scale=dinvl[:, w:w + 1])
            nc.tensor.matmul(ps[:], lhsT=xloc_tiles[w][:], rhs=diag[:],
                             start=True, stop=False)
            mst = msp.tile([128, maxcap1, C], BF16, tag="mst")
            nc.sync.dma_start(
                mst[:, :cap, :],
                ms_d[:, cb * C:(cb + cap) * C].rearrange(
                    "p (j f) -> p j f", f=C))
            # fp8-staged one-hot sel stream (0/1 exact in fp8);
            # the dinv_src multiply upcasts to bf16 for the matmul.
            s1t = spool.tile([128, selw, 128], FP8, tag="s1t")
            nc.sync.dma_start(
                s1t[:, :cap, :],
                sel1s_d[:, cb * 128:(cb + cap) * 128].rearrange(
                    "p (j d) -> p j d", d=128))
            selc = spool.tile([128, selw, 128], BF16, tag="selc")
            nc.vector.tensor_tensor(
                out=selc[:, :cap, :], in0=s1t[:, :cap, :],
                in1=dinvsrc1[:, cb:cb + cap].to_broadcast([128, cap, 128]),
                op=ALU.mult)
            for k in range(cap):
                nc.tensor.matmul(ps[:], lhsT=mst[:, k, :], rhs=selc[:, k, :],
                                 start=False, stop=(k == cap - 1))
            cb += cap
            # epilogue: out1T = relu(W1 @ (dinv_d * psT) + b1)
            s1 = epool.tile([128, 128], BF16, tag="s1")
            nc.vector.tensor_tensor(
                out=s1[:], in0=ps[:],
                in1=dinvdtile[:, w * 128:(w + 1) * 128], op=ALU.mult)
            psh = ppool_h.tile([128, 128], F32, tag="psh")
            nc.tensor.matmul(psh[:], lhsT=w1t[:], rhs=s1[:],
                             start=True, stop=True)
            o1 = epool.tile([128, 128], BF16, tag="o1")
            nc.scalar.activation(o1[:], psh[:], AF.Relu, bias=b1c[:, 0:1])
            # G2 row block: g2 = dinv_d * (out1 @ W2.T), node-major
            psg = ppool_g.tile([128, C], F32, tag="psg")
            nc.tensor.matmul(psg[:], lhsT=o1[:], rhs=w2t[:],
                             start=True, stop=True)
            g2b = gpool.tile([128, C], BF16, tag=f"g2b_{w}")
            nc.scalar.activation(g2b[:], psg[:], AF.Identity,
                                 scale=dinvl[:, w:w + 1])
            if w < NW1:
                nc.sync.dma_start(g2locA[w * 128:(w + 1) * 128, :], g2b[:])
            else:
                nc.sync.dma_start(
                    g2locB[(w - NW1) * 128:(w - NW1 + 1) * 128, :], g2b[:])
            g2b_tiles.append(g2b)
            if w == NW1 - 1:
                # shard-half A is complete on every core at roughly the same
                # time; gather it while the second half is still computing.
                nc.gpsimd.collective_compute(
                    "AllGather", ALU.bypass,
                    replica_groups=[list(range(CORES))],
                    ins=[g2locA[:]], outs=[g2dA[:]])

        # ================= layer 2 (gather, two passes) =================
        # pass A: shard-half-A contributions only (g2dA is available while
        # the second half of L1 still runs); spill partials to SBUF.
        piecesbyw = [([], []) for _ in range(NW)]
        for piece in pieces:
            piecesbyw[piece[0]][piece[1]].append(piece)
        qi = 0
        aggA_tiles = []
        for w in range(NW):
            apieces = piecesbyw[w][0]
            assert apieces
            ps = ppool_w.tile([128, 128], F32, tag="pswA")
            total_k = sum(-(-p[2] // 128) for p in apieces)
            done = 0
            for piece in apieces:
                _, half, nidx, off, ci0 = piece
                msg = emit_gather(piece, qi, msgp, "msg2")
                qi += 1
                nblk = -(-nidx // 128)
                selb = spool.tile([128, selw, 128], BF16, tag="selb")
                nc.vector.tensor_tensor(
                    out=selb[:, :nblk, :], in0=iotav(nblk),
                    in1=rel2[:, ci0:ci0 + nblk].to_broadcast(
                        [128, nblk, 128]),
                    op=ALU.is_equal)
                for k in range(nblk):
                    kk = min(128, nidx - k * 128)
                    done += 1
                    nc.tensor.matmul(ps[:], lhsT=msg[:kk, k, :],
                                     rhs=selb[:kk, k, :],
                                     start=(done == 1),
                                     stop=(done == total_k))
            sA = apool.tile([128, 128], BF16, tag=f"aggA_{w}")
            nc.vector.tensor_copy(sA[:], ps[:])
            aggA_tiles.append(sA)

        nc.gpsimd.collective_compute(
            "AllGather", ALU.bypass,
            replica_groups=[list(range(CORES))],
            ins=[g2locB[:]], outs=[g2dB[:]])

        # pass B: restore partials, add self term + shard-half-B chunks,
        # then epilogue + projection.
        for w in range(NW):
            bpieces = piecesbyw[w][1]
            assert bpieces
            ps = ppool_w.tile([128, 128], F32, tag="pswB")
            nc.tensor.matmul(ps[:], lhsT=g2b_tiles[w][:], rhs=identb[:],
                             start=True, stop=False)
            nc.tensor.matmul(ps[:], lhsT=identb[:], rhs=aggA_tiles[w][:],
                             start=False, stop=False)
            total_k = sum(-(-p[2] // 128) for p in bpieces)
            done = 0
            for piece in bpieces:
                _, half, nidx, off, ci0 = piece
                msg = emit_gather(piece, qi, msgp, "msg2")
                qi += 1
                nblk = -(-nidx // 128)
                selb = spool.tile([128, selw, 128], BF16, tag="selb")
                nc.vector.tensor_tensor(
                    out=selb[:, :nblk, :], in0=iotav(nblk),
                    in1=rel2[:, ci0:ci0 + nblk].to_broadcast(
                        [128, nblk, 128]),
                    op=ALU.is_equal)
                for k in range(nblk):
                    kk = min(128, nidx - k * 128)
                    done += 1
                    nc.tensor.matmul(ps[:], lhsT=msg[:kk, k, :],
                                     rhs=selb[:kk, k, :],
                                     start=False, stop=(done == total_k))
            # epilogue: out2T = relu(dinv_d * psT + b2), then projection
            s2 = epool.tile([128, 128], BF16, tag="s2")
            nc.vector.tensor_tensor(
                out=s2[:], in0=ps[:],
                in1=dinvdtile[:, w * 128:(w + 1) * 128], op=ALU.mult)
            o2 = epool.tile([128, 128], BF16, tag="o2")
            nc.scalar.activation(o2[:], s2[:], AF.Relu, bias=b2c[:, 0:1])
            psp = ppool_p.tile([OUT_C, 128], F32, tag="psp")
            nc.tensor.matmul(psp[:], lhsT=wpt[:], rhs=o2[:],
                             start=True, stop=True)
            ofr = epool.tile([OUT_C, 128], F32, tag="ofr")
            nc.scalar.activation(ofr[:], psp[:], AF.Relu, bias=bpc[:, 0:1])
            nc.sync.dma_start(out_d[:, w * 128:(w + 1) * 128], ofr[:])

    nc.compile()
    return nc


def _make_in_maps(cfg, sched, per_core, W1, b1, W2, b2, Wp, bp):
    w1t = np.ascontiguousarray(np.asarray(W1, np.float32).T).astype(NP_BF16)
    w2t = np.ascontiguousarray(np.asarray(W2, np.float32).T).astype(NP_BF16)
    wpt = np.ascontiguousarray(np.asarray(Wp, np.float32).T).astype(NP_BF16)
    b1col = np.ascontiguousarray(np.asarray(b1, np.float32)[:, None])
    b2col = np.ascontiguousarray(np.asarray(b2, np.float32)[:, None])
    bpcol = np.ascontiguousarray(np.asarray(bp, np.float32)[:, None])
    iotar = np.tile(np.arange(128, dtype=np.float32)[None],
                    (128, 32)).astype(NP_BF16)
    identb = np.eye(128, dtype=np.float32).astype(NP_BF16)
    base = dict(w1t=w1t, w2t=w2t, wpt=wpt, b1c=b1col, b2c=b2col, bpc=bpcol,
                iotar=iotar, identb=identb)
    in_maps = []
    for c in range(cfg.CORES):
        pc = per_core[c]
        m = dict(base)
        m["msgstream"] = pc["msgstream"]
        m["sel1s"] = pc["sel1s"]
        m["degsrc1"] = pc["degsrc1"]
        m["idxa"] = pc["idxa"] if pc["idxa"].size else np.zeros((128, 16), np.int16)
        m["idxb"] = pc["idxb"] if pc["idxb"].size else np.zeros((128, 16), np.int16)
        m["rel2"] = pc["rel2"]
        m["degl"] = pc["degl"]
        m["degdt"] = pc["degdt"]
        m["xloc"] = pc["xloc"]
        in_maps.append(m)
    return in_maps


def _run(inputs, cfg=None, trace=False, tmpdir=None, verbose=True):
    import time
    t0 = time.time()
    def _log(msg):
        if verbose:
            print(f"[kernel {time.time()-t0:7.1f}s] {msg}", flush=True)
    cfg = cfg or CFG
    sched, per_core = _host_prep(cfg, inputs["x"], inputs["edge_index"])
    _log("host prep done")
    nc = _build_nc(cfg, sched)
    _log("build+compile done")
    in_maps = _make_in_maps(cfg, sched, per_core,
                            inputs["W1"], inputs["b1"], inputs["W2"],
                            inputs["b2"], inputs["Wp"], inputs["bp"])
    _log("in_maps done")
    core_ids = list(range(cfg.CORES))
    if trace:
        bass_utils.run_bass_kernel_spmd(nc, in_maps, core_ids=core_ids,
                                        trace=False)
        _log("warmup run done")
    res = bass_utils.run_bass_kernel_spmd(nc, in_maps, core_ids=core_ids,
                                          trace=trace, tmpdir=tmpdir)
    _log("run done")
    out = np.empty((cfg.N, cfg.OUT_C), np.float32)
    for c in range(cfg.CORES):
        out[c * cfg.S:(c + 1) * cfg.S] = res.results[c]["out"][:, :cfg.S].T
    return out, res


def kernel(**inputs):
    out, _ = _run(inputs)
    return out
